# revision 11
# baseline (speedup 1.0000x reference)
"""Llama attention (B=1, S=2048, HID=2048, 16 heads x d=128) on 8 TRN2 NeuronCores.

Sharding: tensor-parallel over heads — 2 heads per core. Each core:
  - Q/K/V projections for its 2 heads in transposed layout Q^T/K^T/V^T [d, s]
    (lhsT = host-pretransposed weight chunk, rhs = host-pretransposed X^T)
  - RoPE applied on [d, s] with host-precomputed cos/sin tables; the
    rotate-half is a partition-shifted copy done by DMA (PSUM -> SBUF), and the
    sign of the rotation plus the 1/sqrt(d) score scale are folded into the
    tables.
  - scores are computed TRANSPOSED (S^T[k, q] = K Q^T) so softmax needs no
    P-transposes: exp on ACT (no max subtraction needed: |scores| <~ 6 because
    inputs are unit-variance), row sums via ones-vector matmuls on PE,
    causal masking at block granularity (upper strips skipped entirely,
    one [128,128] triangular multiplicative mask on diagonal blocks).
  - O^T = (V^T P^T)^T accumulated per 512-wide q chunk, normalized by the
    broadcast reciprocal row sums.
  - o_proj partial: out_c = O_heads @ Wo[:, heads].T  [2048, 2048]
Host sums the 8 partial outputs (the TP reduce).

Matmuls run in bf16 (f32 accumulation in PSUM); everything else f32.
"""

import math
import sys

import numpy as np

sys.path.insert(0, "/opt/trn_rl_repo")

import ml_dtypes

BF16 = ml_dtypes.bfloat16
S = 2048
HID = 2048
NH = 16
D = 128
NCORES = 8
HPC = 2            # heads per core
M = HPC * D        # 256 = per-core projection width
ROPE_THETA = 10000.0
RSQ = 1.0 / math.sqrt(D)

_BUILT = None
LAST_RESULTS = None  # BassKernelResults of the most recent run (for profiling)


def _build():
    from contextlib import ExitStack

    import concourse.bacc as bacc
    import concourse.bass as bass
    import concourse.mybir as mybir
    import concourse.tile as tile

    dt = mybir.dt
    f32, bf = dt.float32, dt.bfloat16
    MUL, ADD = mybir.AluOpType.mult, mybir.AluOpType.add
    EXP = mybir.ActivationFunctionType.Exp

    nc = bacc.Bacc(
        "TRN2",
        target_bir_lowering=False,
        debug=False,
        enable_asserts=True,
        num_devices=NCORES,
    )

    xt_d = nc.dram_tensor("xt", [HID, S], bf, kind="ExternalInput").ap()
    wq_d = nc.dram_tensor("wq", [HID, M], bf, kind="ExternalInput").ap()
    wk_d = nc.dram_tensor("wk", [HID, M], bf, kind="ExternalInput").ap()
    wv_d = nc.dram_tensor("wv", [HID, M], bf, kind="ExternalInput").ap()
    wo_d = nc.dram_tensor("wo", [M, S], bf, kind="ExternalInput").ap()
    cq_d = nc.dram_tensor("cosq", [D, S], bf, kind="ExternalInput").ap()
    sq_d = nc.dram_tensor("sinq", [D, S], bf, kind="ExternalInput").ap()
    ck_d = nc.dram_tensor("cosk", [D, S], bf, kind="ExternalInput").ap()
    sk_d = nc.dram_tensor("sink", [D, S], bf, kind="ExternalInput").ap()
    tri_d = nc.dram_tensor("triu", [D, D], bf, kind="ExternalInput").ap()
    one_d = nc.dram_tensor("ones", [D, 1], bf, kind="ExternalInput").ap()
    id_d = nc.dram_tensor("ident", [D, D], bf, kind="ExternalInput").ap()
    out_d = nc.dram_tensor("out", [S, S], bf, kind="ExternalOutput").ap()

    with ExitStack() as ctx:
        tc = ctx.enter_context(tile.TileContext(nc))

        p_xt = ctx.enter_context(tc.tile_pool(name="xt", bufs=16))
        p_w = ctx.enter_context(tc.tile_pool(name="wts", bufs=16))
        p_wo = ctx.enter_context(tc.tile_pool(name="wo", bufs=2))
        p_tbl = ctx.enter_context(tc.tile_pool(name="tbl", bufs=4))
        p_cst = ctx.enter_context(tc.tile_pool(name="cst", bufs=1))
        p_act = ctx.enter_context(tc.tile_pool(name="act", bufs=6))
        p_vsd = ctx.enter_context(tc.tile_pool(name="vsd", bufs=16))
        p_strip = ctx.enter_context(tc.tile_pool(name="strip", bufs=16))
        p_tmp = ctx.enter_context(tc.tile_pool(name="tmp", bufs=3))
        p_rot = ctx.enter_context(tc.tile_pool(name="rot", bufs=2))
        p_out = ctx.enter_context(tc.tile_pool(name="outst", bufs=2))
        p_rc = ctx.enter_context(tc.tile_pool(name="rc", bufs=2))
        p_rcb = ctx.enter_context(tc.tile_pool(name="rcb", bufs=2))
        p_ps = ctx.enter_context(tc.tile_pool(name="ps", bufs=4, space="PSUM"))

        # ---- constants / weights / tables ----
        tri = p_cst.tile([D, D], bf, tag="tri", name="tri")
        nc.gpsimd.dma_start(tri[:], tri_d[:])
        one = p_cst.tile([D, 1], bf, tag="one", name="one")
        nc.sync.dma_start(one[:], one_d[:])
        idn = p_cst.tile([D, D], bf, tag="idn", name="idn")
        nc.sync.dma_start(idn[:], id_d[:])

        tbl = {}
        for nm, dram in (("cq", cq_d), ("sq", sq_d), ("ck", ck_d), ("sk", sk_d)):
            t = p_tbl.tile([D, S], bf, tag="tbl", name=f"tbl_{nm}")
            # chunked so downstream consumers wait on few DMA queues each
            for qc in range(4):
                nc.gpsimd.dma_start(
                    t[:, qc * 512 : (qc + 1) * 512],
                    dram[:, qc * 512 : (qc + 1) * 512],
                )
            tbl[nm] = t

        wts = {}
        for nm, dram in (("q", wq_d), ("k", wk_d), ("v", wv_d)):
            wts[nm] = []
            for kk in range(16):
                w = p_w.tile([D, M], bf, tag=f"w{nm}", name=f"w{nm}{kk}")
                nc.sync.dma_start(w[:], dram[kk * 128 : (kk + 1) * 128, :])
                wts[nm].append(w)
        wo_sb = []
        for h in range(HPC):
            w = p_wo.tile([D, S], bf, tag="wo", name=f"wo{h}")
            nc.sync.dma_start(w[:], wo_d[h * 128 : (h + 1) * 128, :])
            wo_sb.append(w)

        # DVE wait-absorbers: TT instructions can carry only ONE sync wait,
        # so prime the DVE vector clock with every table/mask DMA here.
        scrb = p_cst.tile([1, 32], bf, tag="scrb", name="scrb")
        j = 2
        for t in tbl.values():
            for qc in range(4):
                nc.vector.tensor_copy(out=scrb[0:1, j : j + 1], in_=t[0:1, qc * 512 : qc * 512 + 1])
                j += 1
        nc.vector.tensor_copy(out=scrb[0:1, 1:2], in_=tri[0:1, 0:1])

        qT = [p_act.tile([D, S], bf, tag="act", name=f"qT{h}") for h in range(HPC)]
        kT = [p_act.tile([D, S], bf, tag="act", name=f"kT{h}") for h in range(HPC)]
        vT = [p_act.tile([D, S], bf, tag="act", name=f"vT{h}") for h in range(HPC)]

        # ---- projections (+ RoPE for q/k) ----
        xts = []
        for kk in range(16):
            x = p_xt.tile([D, S], bf, tag="xt", name=f"xt_{kk}")
            nc.sync.dma_start(x[:], xt_d[kk * 128 : (kk + 1) * 128, :])
            xts.append(x)
        for nm, hh in (
            ("q", 0), ("q", 1), ("k", 0), ("k", 1), ("v", 0), ("v", 1),
        ):
            for half in range(2):
                cols = slice(half * 1024, half * 1024 + 1024)
                pj = p_ps.tile([D, 1024], f32, tag="ps", name=f"pj_{half}_{nm}{hh}")
                for kk in range(16):
                    lhsT = wts[nm][kk][:, hh * 128 : (hh + 1) * 128]
                    for sc in range(2):
                        g = half * 1024 + sc * 512
                        nc.tensor.matmul(
                            pj[:, sc * 512 : (sc + 1) * 512],
                            lhsT,
                            xts[kk][:, g : g + 512],
                            start=(kk == 0),
                            stop=(kk == 15),
                        )
                if nm == "v":
                    nc.scalar.copy(out=vT[hh][:, cols], in_=pj[:])
                    continue
                ct = tbl["cq" if nm == "q" else "ck"]
                st = tbl["sq" if nm == "q" else "sk"]
                rot = p_rot.tile([D, 1024], f32, tag="rot", name=f"rot_{half}_{nm}{hh}")
                # rotate-half = partition shift by 64 (sign folded into sin table);
                # 64-partition DVE ops may write the opposite partition half
                nc.vector.tensor_copy(out=rot[0:64, :], in_=pj[64:128, :])
                nc.vector.tensor_copy(out=rot[64:128, :], in_=pj[0:64, :])
                t1 = p_tmp.tile([D, 1024], f32, tag="tmp", name=f"t1_{half}_{nm}{hh}")
                t2 = p_tmp.tile([D, 1024], f32, tag="tmp", name=f"t2_{half}_{nm}{hh}")
                dst = (qT if nm == "q" else kT)[hh]
                # 512-col sub-ops: keep per-instruction sync-wait count low
                for sc in range(2):
                    a, b = sc * 512, (sc + 1) * 512
                    ga, gb = half * 1024 + a, half * 1024 + b
                    nc.vector.tensor_tensor(t1[:, a:b], pj[:, a:b], ct[:, ga:gb], MUL)
                    nc.vector.tensor_tensor(t2[:, a:b], rot[:, a:b], st[:, ga:gb], MUL)
                    nc.vector.tensor_tensor(dst[:, ga:gb], t1[:, a:b], t2[:, a:b], ADD)

        # ---- V^T -> V[s, d] via PE transposes ----
        v_sd = [p_vsd.tile([D, M], bf, tag="vsd", name=f"vsd{i}") for i in range(16)]
        for hh in range(HPC):
            for kb in range(16):
                vt_ps = p_ps.tile([D, D], bf, tag="ps", name=f"vtps_{hh}_{kb}")
                nc.tensor.transpose(
                    vt_ps[:], vT[hh][:, kb * 128 : (kb + 1) * 128], idn[:]
                )
                nc.scalar.copy(
                    out=v_sd[kb][:, hh * 128 : (hh + 1) * 128], in_=vt_ps[:]
                )

        # ---- attention (scores transposed; causal at block granularity) ----
        oT = [p_act.tile([D, S], bf, tag="act", name=f"oT{h}") for h in range(HPC)]
        for hh in range(HPC):
            for c in range(4):
                kbn = 4 * (c + 1)
                strips = []
                for kb in range(kbn):
                    stp = p_strip.tile([D, 512], bf, tag="strip", name=f"stp_{hh}_{c}_{kb}")
                    sT = p_ps.tile([D, 512], f32, tag="ps", name=f"sT_{hh}_{c}_{kb}")
                    kslice = kT[hh][:, kb * 128 : (kb + 1) * 128]
                    if kb >= 4 * c:
                        off = (kb - 4 * c) * 128
                        nc.tensor.matmul(
                            sT[:, off:512],
                            kslice,
                            qT[hh][:, c * 512 + off : (c + 1) * 512],
                            start=True,
                            stop=True,
                        )
                        nc.scalar.activation(stp[:, off:512], sT[:, off:512], EXP)
                        if off:
                            nc.vector.memset(stp[:, 0:off], 0.0)
                        nc.vector.tensor_tensor(
                            stp[:, off : off + 128],
                            stp[:, off : off + 128],
                            tri[:],
                            MUL,
                        )
                    else:
                        nc.tensor.matmul(
                            sT[:],
                            kslice,
                            qT[hh][:, c * 512 : (c + 1) * 512],
                            start=True,
                            stop=True,
                        )
                        nc.scalar.activation(stp[:], sT[:], EXP)
                    strips.append(stp)
                ot = p_ps.tile([D, 512], f32, tag="ps", name=f"ot_{hh}_{c}")
                for kb in range(kbn):
                    nc.tensor.matmul(
                        ot[:],
                        v_sd[kb][:, hh * 128 : (hh + 1) * 128],
                        strips[kb][:],
                        start=(kb == 0),
                        stop=(kb == kbn - 1),
                    )
                rs = p_ps.tile([1, 512], f32, tag="ps", name=f"rs_{hh}_{c}")
                for kb in range(kbn):
                    nc.tensor.matmul(
                        rs[:], one[:], strips[kb][:],
                        start=(kb == 0), stop=(kb == kbn - 1),
                    )
                rc = p_rc.tile([1, 512], f32, tag="rc", name=f"rc_{hh}_{c}")
                nc.vector.reciprocal(rc[:], rs[:])
                rcb = p_rcb.tile([D, 512], f32, tag="rcb", name=f"rcb_{hh}_{c}")
                nc.gpsimd.partition_broadcast(rcb[:], rc[:])
                nc.vector.tensor_tensor(
                    oT[hh][:, c * 512 : (c + 1) * 512], ot[:], rcb[:], MUL
                )

        # ---- o_proj partial: out = sum_h O_h @ Wo_h^T ----
        for sb in range(16):
            ost = p_out.tile([D, S], bf, tag="outst", name=f"ost_{sb}")
            # 1-elem ACT touch absorbs the slot's WAR-on-out-DMA wait
            nc.scalar.copy(out=ost[0:1, 0:1], in_=scrb[0:1, 0:1])
            for oc in range(4):
                op = p_ps.tile([D, 512], f32, tag="ps", name=f"op_{sb}_{oc}")
                for hh in range(HPC):
                    nc.tensor.matmul(
                        op[:],
                        oT[hh][:, sb * 128 : (sb + 1) * 128],
                        wo_sb[hh][:, oc * 512 : (oc + 1) * 512],
                        start=(hh == 0),
                        stop=(hh == HPC - 1),
                    )
                nc.scalar.copy(out=ost[:, oc * 512 : (oc + 1) * 512], in_=op[:])
            nc.scalar.dma_start(out_d[sb * 128 : (sb + 1) * 128, :], ost[:])

    nc.compile()
    return nc


def _prep_inputs(hidden_states, Wq, Wk, Wv, Wo, position_ids):
    X = np.asarray(hidden_states, dtype=np.float32).reshape(S, HID)
    xt_b = np.ascontiguousarray(X.T).astype(BF16)
    pos = np.asarray(position_ids).reshape(S).astype(np.float32)
    inv_freq = (
        1.0 / (ROPE_THETA ** (np.arange(0, D, 2, dtype=np.float32) / np.float32(D)))
    ).astype(np.float32)
    fr = inv_freq[:, None] * pos[None, :]  # [64, S]
    cos = np.cos(fr).astype(np.float32)
    sin = np.sin(fr).astype(np.float32)
    cos2 = np.concatenate([cos, cos], axis=0)  # [128, S]
    sin2 = np.concatenate([-sin, sin], axis=0)  # sign of rotate-half folded in
    tri = np.triu(np.ones((D, D), dtype=np.float32)).astype(BF16)
    ones = np.ones((D, 1), dtype=BF16)
    ident = np.eye(D, dtype=np.float32).astype(BF16)
    Wqf = np.asarray(Wq, np.float32)
    Wkf = np.asarray(Wk, np.float32)
    Wvf = np.asarray(Wv, np.float32)
    Wof = np.asarray(Wo, np.float32)

    cosq = np.ascontiguousarray(cos2 * np.float32(RSQ)).astype(BF16)
    sinq = np.ascontiguousarray(sin2 * np.float32(RSQ)).astype(BF16)
    cosk = np.ascontiguousarray(cos2).astype(BF16)
    sink = np.ascontiguousarray(sin2).astype(BF16)

    in_maps = []
    for c in range(NCORES):
        J = slice(c * M, (c + 1) * M)
        in_maps.append(
            dict(
                xt=xt_b,
                wq=np.ascontiguousarray(Wqf[J].T).astype(BF16),
                wk=np.ascontiguousarray(Wkf[J].T).astype(BF16),
                wv=np.ascontiguousarray(Wvf[J].T).astype(BF16),
                wo=np.ascontiguousarray(Wof[:, J].T).astype(BF16),
                cosq=cosq,
                sinq=sinq,
                cosk=cosk,
                sink=sink,
                triu=tri,
                ones=ones,
                ident=ident,
            )
        )
    return in_maps


def kernel(hidden_states, Wq, Wk, Wv, Wo, attention_mask, position_ids):
    global _BUILT, LAST_RESULTS
    if _BUILT is None:
        _BUILT = _build()
    nc = _BUILT
    from concourse.bass_utils import run_bass_kernel_spmd

    in_maps = _prep_inputs(hidden_states, Wq, Wk, Wv, Wo, position_ids)
    res = run_bass_kernel_spmd(nc, in_maps, core_ids=list(range(NCORES)))
    LAST_RESULTS = res
    acc = np.zeros((S, S), np.float32)
    for r in res.results:
        acc += np.asarray(r["out"]).astype(np.float32)
    return acc.reshape(1, S, S)
